# revision 35
# baseline (speedup 1.0000x reference)
"""Trainium2 Bass kernel for nn_AdaptiveMoELLM (2-layer MoE transformer with
lightning-indexer top-K attention and top-2-of-8 MoE routing, vocab head).

Distribution over 8 NeuronCores:
  - tokens (B*S = 2048) sharded 256/core for attention/norms/routing
    (cores 0-3 = batch 0, cores 4-7 = batch 1; AllGather of normalized
    activations within each 4-core batch group feeds full-sequence K/V)
  - experts sharded 1/core (dense token processing, gate-weighted,
    ReduceScatter returns summed per-token rows to their owners; gate
    columns routed to expert owners via AllToAll)
  - vocab projection column-sharded 4000/core after a final AllGather

The residual/x path runs in full fp32 (PE f32 matmuls) so that the discrete
router top-2 decisions match the fp32 reference exactly; the indexer scores
and the final vocab projection run in bf16 (their errors are continuous /
empirically mask-exact).  Host side precomputes embeddings and folds
LayerNorm gains + attention scale + indexer head magnitudes into weights.
"""

import numpy as np
import ml_dtypes

import concourse.bass as bass
import concourse.bacc as bacc
import concourse.mybir as mybir
import concourse.tile as tile
from concourse.bass_utils import run_bass_kernel_spmd

F32 = mybir.dt.float32
F32R = mybir.dt.float32r
BF16 = mybir.dt.bfloat16
I32 = mybir.dt.int32
AF = mybir.ActivationFunctionType
ALU = mybir.AluOpType
AX = mybir.AxisListType

L, D, H, DH, HI, DI, F, E = 2, 512, 8, 64, 4, 64, 2048, 8
V, S, B, K, TOPK_E = 32000, 1024, 2, 256, 2
NC = 8
TPC = 256
T = B * S
VPC = V // NC
EPS = 1e-5
N_ITERS = 23

bf16 = ml_dtypes.bfloat16


def _build(signs, dbg=False):
    nc = bacc.Bacc(None, num_devices=NC, debug=False, target_bir_lowering=False)

    def param(name, shape, dt):
        return nc.declare_dram_parameter(name, list(shape), dt, isOutput=False)

    x0_p = param("x0", [TPC, D], F32)
    c01_p = param("c01", [TPC, S], F32)
    idxq_p = param("idxq", [L, D, HI * DI], F32)
    idxk_p = param("idxk", [L, D, HI * DI], F32)
    wqkv_p = param("wqkv", [L, 4, D, D], F32)
    rw_p = param("rw", [L, D, E], F32)
    w1_p = param("w1", [L, 2, D, F], BF16)
    w2_p = param("w2", [L, 2, F, D], BF16)
    outw_p = param("outw", [D, VPC], BF16)
    idb_p = param("idb", [128, 128], BF16)
    idf_p = param("idf", [128, 128], F32)
    out_p = nc.declare_dram_parameter("out", [T, VPC], F32, isOutput=True)

    with tile.TileContext(nc) as tc:
        with (
            tc.tile_pool(name="cst", bufs=1) as cst,
            tc.tile_pool(name="wrk", bufs=2) as wrk,
            tc.tile_pool(name="sml", bufs=4) as sml,
            tc.tile_pool(name="ps", bufs=4, space="PSUM") as ps,
            tc.tile_pool(name="pst", bufs=2, space="PSUM") as pst,
            tc.tile_pool(name="dr", bufs=1, space="DRAM") as dr,
        ):
            # ---------------- persistent loads ----------------
            ident = cst.tile([128, 128], BF16)
            nc.sync.dma_start(ident[:], idb_p[:])
            identf = cst.tile([128, 128], F32)
            nc.sync.dma_start(identf[:], idf_p[:])
            x_own = cst.tile([128, 2, D], F32)
            nc.sync.dma_start(
                x_own[:], x0_p.rearrange("(t p) d -> p t d", p=128))
            c01 = cst.tile([128, 2, S], F32)
            nc.sync.dma_start(c01[:], c01_p.rearrange("(t p) k -> p t k", p=128))
            vals = cst.tile([128, 2, S], F32)
            ind = cst.tile([128, 2, S], BF16)
            indT = cst.tile([128, 8, TPC], BF16)

            def mm_ps(shape, pool=None, tag="mm", bufs=None):
                pool = pool or ps
                return pool.tile(shape, F32, tag=tag, bufs=bufs,
                                 name=f"ps_{tag}_{nc.next_id()}")

            def dump(name, ap):
                if not dbg:
                    return
                t = nc.declare_dram_parameter(
                    "dbg_" + name, list(ap.shape), ap.dtype, isOutput=True)
                nc.sync.dma_start(t[:], ap)

            def transpose_128(dst, src, dtype=F32):
                pt = pst.tile([128, 128], dtype, tag="tr",
                              name=f"pt_{nc.next_id()}")
                nc.tensor.transpose(
                    pt[:], src, ident[:] if dtype == BF16 else identf[:])
                nc.vector.tensor_copy(out=dst, in_=pt[:])

            def normalize(src_qt, dst_qt):
                """LayerNorm without affine (folded into consumers). f32."""
                ssum = sml.tile([128, 1], F32, tag="ln_s",
                                name=f"lns_{nc.next_id()}")
                nc.vector.tensor_reduce(
                    out=ssum[:], in_=src_qt, axis=AX.X, op=ALU.add)
                negmean = sml.tile([128, 1], F32, tag="ln_m",
                                   name=f"lnm_{nc.next_id()}")
                nc.vector.tensor_scalar(
                    out=negmean[:], in0=ssum[:], scalar1=-1.0 / D,
                    scalar2=None, op0=ALU.mult)
                xc = wrk.tile([128, D], F32, tag="ln_xc", bufs=1,
                              name=f"lnxc_{nc.next_id()}")
                var = sml.tile([128, 1], F32, tag="ln_v",
                               name=f"lnv_{nc.next_id()}")
                nc.vector.scalar_tensor_tensor(
                    out=xc[:], in0=src_qt, scalar=negmean[:], in1=src_qt,
                    op0=ALU.add, op1=ALU.bypass)
                sq = wrk.tile([128, D], F32, tag="ln_sq", bufs=1,
                              name=f"lnsq_{nc.next_id()}")
                nc.vector.scalar_tensor_tensor(
                    out=sq[:], in0=xc[:], scalar=1.0, in1=xc[:],
                    op0=ALU.mult, op1=ALU.mult, accum_out=var[:])
                vmean = sml.tile([128, 1], F32, tag="ln_vm",
                                 name=f"lnvm_{nc.next_id()}")
                nc.vector.tensor_scalar(
                    out=vmean[:], in0=var[:], scalar1=1.0 / D, scalar2=EPS,
                    op0=ALU.mult, op1=ALU.add)
                std = sml.tile([128, 1], F32, tag="ln_sd",
                               name=f"lnsd_{nc.next_id()}")
                nc.scalar.activation(std[:], vmean[:], AF.Sqrt)
                rstd = sml.tile([128, 1], F32, tag="ln_r",
                                name=f"lnr_{nc.next_id()}")
                nc.vector.reciprocal(rstd[:], std[:])
                # one Newton step: r' = r*(1.5 - 0.5*v*r^2)  (Sqrt ULP hedge)
                r2 = sml.tile([128, 1], F32, tag="ln_r2",
                              name=f"lnr2_{nc.next_id()}")
                nc.vector.tensor_tensor(out=r2[:], in0=rstd[:], in1=rstd[:],
                                        op=ALU.mult)
                vr2 = sml.tile([128, 1], F32, tag="ln_vr",
                               name=f"lnvr_{nc.next_id()}")
                nc.vector.tensor_tensor(out=vr2[:], in0=vmean[:], in1=r2[:],
                                        op=ALU.mult)
                nc.vector.tensor_scalar(
                    out=vr2[:], in0=vr2[:], scalar1=-0.5, scalar2=1.5,
                    op0=ALU.mult, op1=ALU.add)
                nc.vector.tensor_tensor(out=rstd[:], in0=rstd[:], in1=vr2[:],
                                        op=ALU.mult)
                nc.vector.tensor_scalar(
                    out=dst_qt, in0=xc[:], scalar1=rstd[:], scalar2=None,
                    op0=ALU.mult)

            # =======================================================
            for l in range(L):
                xh = wrk.tile([128, 2, D], F32, tag="xh", bufs=1,
                              name=f"xh_{l}")
                for qt in range(2):
                    normalize(x_own[:, qt, :], xh[:, qt, :])

                with (
                    tc.tile_pool(name=f"attn{l}", bufs=1) as ab,
                    tc.tile_pool(name=f"aops{l}", bufs=2, space="PSUM") as aops,
                ):
                    idxq_sb = ab.tile([128, 4, HI * DI], F32, tag="idxq",
                                      name=f"idxq_{l}")
                    nc.sync.dma_start(
                        idxq_sb[:],
                        idxq_p[l].rearrange("(d p) n -> p d n", p=128))
                    idxk_sb = ab.tile([128, 4, HI * DI], F32, tag="idxk",
                                      name=f"idxk_{l}")
                    nc.sync.dma_start(
                        idxk_sb[:],
                        idxk_p[l].rearrange("(d p) n -> p d n", p=128))
                    wqkv_sb = ab.tile([128, 4, 4, D], F32, tag="wqkv",
                                      name=f"wqkv_{l}")
                    nc.sync.dma_start(
                        wqkv_sb[:],
                        wqkv_p[l].rearrange("m (d p) n -> p m d n", p=128))

                    hT_own = ab.tile([128, 4, TPC], F32, tag="hT_own",
                                     name=f"hTo_{l}")
                    for qt in range(2):
                        for dt in range(4):
                            transpose_128(
                                hT_own[:, dt, qt * 128:(qt + 1) * 128],
                                xh[:, qt, dt * 128:(dt + 1) * 128])
                    dump(f"hTo{l}", hT_own[:])
                    # q-side projections only need local hT_own; issue them
                    # early so PE works while AG1 is in flight
                    qiT_l = []
                    for hp in range(HI // 2):
                        qiT = ab.tile([128, TPC], F32, tag="qiT", bufs=2,
                                      name=f"qiT_{nc.next_id()}")
                        pq = mm_ps([128, TPC])
                        for dt in range(4):
                            nc.tensor.matmul(
                                pq[:],
                                idxq_sb[:, dt, hp * 128:(hp + 1) * 128],
                                hT_own[:, dt, :], start=dt == 0,
                                stop=dt == 3)
                        nc.scalar.copy(qiT[:], pq[:])
                        qiT_l.append(qiT)
                    qhT_l = []
                    for hp in range(H // 2):
                        qhT = ab.tile([128, TPC], F32, tag="qhT", bufs=4,
                                      name=f"qhT_{nc.next_id()}")
                        pq = mm_ps([128, TPC])
                        for dt in range(4):
                            nc.tensor.matmul(
                                pq[:],
                                wqkv_sb[:, 0, dt, hp * 128:(hp + 1) * 128],
                                hT_own[:, dt, :], start=dt == 0, stop=dt == 3)
                        nc.scalar.copy(qhT[:], pq[:])
                        qhT_l.append(qhT)
                    ag1_in = dr.tile([D, TPC], F32, tag="ag1i",
                                     name=f"ag1i_{l}")
                    nc.sync.dma_start(
                        ag1_in.rearrange("(d p) t -> p d t", p=128), hT_own[:])
                    ag1_out = dr.tile([4 * D, TPC], F32, tag="ag1o",
                                      name=f"ag1o_{l}")
                    nc.gpsimd.collective_compute(
                        "AllGather", ALU.bypass,
                        ins=[ag1_in[:]], outs=[ag1_out[:]],
                        replica_groups=[[0, 1, 2, 3], [4, 5, 6, 7]])
                    hT_b = ab.tile([128, 4, S], F32, tag="hT_b",
                                   name=f"hTb_{l}")
                    for r in range(4):
                        nc.sync.dma_start(
                            hT_b[:, :, r * TPC:(r + 1) * TPC],
                            ag1_out[r * D:(r + 1) * D].rearrange(
                                "(d p) t -> p d t", p=128))
                    dump(f"hTb{l}", hT_b[:])

                    # ---- lightning indexer scores -> vals ----
                    for qt in range(2):
                        for ch in range(2):
                            nc.vector.tensor_scalar(
                                out=vals[:, qt, ch * 512:(ch + 1) * 512],
                                in0=c01[:, qt, ch * 512:(ch + 1) * 512],
                                scalar1=1e9, scalar2=-1e9,
                                op0=ALU.mult, op1=ALU.add)
                    for hp in range(HI // 2):
                        qiT = qiT_l[hp]
                        kiT = ab.tile([128, S], F32, tag="kiT", bufs=2,
                                      name=f"kiT_{nc.next_id()}")
                        for ch in range(2):
                            pk = mm_ps([128, 512])
                            for dt in range(4):
                                nc.tensor.matmul(
                                    pk[:],
                                    idxk_sb[:, dt, hp * 128:(hp + 1) * 128],
                                    hT_b[:, dt, ch * 512:(ch + 1) * 512],
                                    start=dt == 0, stop=dt == 3)
                            nc.scalar.copy(
                                kiT[:, ch * 512:(ch + 1) * 512], pk[:])
                        for hh in range(2):
                            h = hp * 2 + hh
                            for qt in range(2):
                                for ch in range(2):
                                    pv = mm_ps([128, 512])
                                    nc.tensor.matmul(
                                        pv[:],
                                        qiT[hh * 64:(hh + 1) * 64,
                                            qt * 128:(qt + 1) * 128],
                                        kiT[hh * 64:(hh + 1) * 64,
                                            ch * 512:(ch + 1) * 512],
                                        start=True, stop=True)
                                    rl = ab.tile([128, 512], F32, tag="rl",
                                                 bufs=2,
                                                 name=f"rl_{nc.next_id()}")
                                    nc.scalar.activation(rl[:], pv[:], AF.Relu)
                                    dst = vals[:, qt, ch * 512:(ch + 1) * 512]
                                    nc.vector.scalar_tensor_tensor(
                                        out=dst, in0=rl[:],
                                        scalar=float(signs[l][h]), in1=dst,
                                        op0=ALU.mult, op1=ALU.add)

                    # ---- top-K threshold: per-row binary search ----
                    lo = sml.tile([128, 2], F32, tag="lo", name=f"lo_{l}")
                    hi = sml.tile([128, 2], F32, tag="hi", name=f"hi_{l}")
                    for qt in range(2):
                        nc.vector.tensor_reduce(
                            out=hi[:, qt:qt + 1], in_=vals[:, qt, :],
                            axis=AX.X, op=ALU.max)
                        msk = ab.tile([128, S], F32, tag="msk", bufs=1,
                                      name=f"msk_{nc.next_id()}")
                        nc.vector.tensor_tensor(
                            out=msk[:], in0=vals[:, qt, :], in1=c01[:, qt, :],
                            op=ALU.mult)
                        nc.vector.tensor_reduce(
                            out=lo[:, qt:qt + 1], in_=msk[:], axis=AX.X,
                            op=ALU.min)
                    counts = sml.tile([128, 2], F32, tag="cnt",
                                      name=f"cnt_{l}")
                    for it in range(N_ITERS):
                        mid = sml.tile([128, 2], F32, tag="mid",
                                       name=f"mid_{nc.next_id()}")
                        nc.vector.tensor_tensor(
                            out=mid[:], in0=lo[:], in1=hi[:], op=ALU.add)
                        nc.vector.tensor_scalar(
                            out=mid[:], in0=mid[:], scalar1=0.5, scalar2=None,
                            op0=ALU.mult)
                        for qt in range(2):
                            junk = ab.tile([128, S], BF16, tag="junk", bufs=1,
                                           name=f"jk_{nc.next_id()}")
                            nc.vector.tensor_scalar(
                                out=junk[:], in0=vals[:, qt, :],
                                scalar1=mid[:, qt:qt + 1], scalar2=0.0,
                                op0=ALU.is_ge, op1=ALU.add,
                                accum_out=counts[:, qt:qt + 1])
                        hit = sml.tile([128, 2], I32, tag="hit",
                                       name=f"hit_{nc.next_id()}")
                        nc.vector.tensor_scalar(
                            out=hit[:], in0=counts[:], scalar1=float(K),
                            scalar2=None, op0=ALU.is_ge)
                        nc.vector.copy_predicated(lo[:], hit[:], mid[:])
                        nhit = sml.tile([128, 2], I32, tag="nhit",
                                        name=f"nh_{nc.next_id()}")
                        nc.vector.tensor_scalar(
                            out=nhit[:], in0=counts[:], scalar1=float(K),
                            scalar2=None, op0=ALU.is_lt)
                        nc.vector.copy_predicated(hi[:], nhit[:], mid[:])

                    for qt in range(2):
                        nc.vector.tensor_scalar(
                            out=ind[:, qt, :], in0=vals[:, qt, :],
                            scalar1=lo[:, qt:qt + 1], scalar2=None,
                            op0=ALU.is_ge)
                    dump(f"vals{l}", vals[:])
                    dump(f"lo{l}", lo[:])
                    dump(f"ind{l}", ind[:])
                    for qt in range(2):
                        for kt in range(8):
                            transpose_128(
                                indT[:, kt, qt * 128:(qt + 1) * 128],
                                ind[:, qt, kt * 128:(kt + 1) * 128],
                                dtype=BF16)

                    # ---- attention (fp32) ----
                    v_sb = ab.tile([128, 8, H, DH + 1], F32, tag="v_sb",
                                   name=f"v_{l}")
                    nc.vector.memset(v_sb[:, :, :, DH:DH + 1], 1.0)
                    for kt in range(8):
                        pvv = mm_ps([128, 512])
                        for dt in range(4):
                            nc.tensor.matmul(
                                pvv[:], hT_b[:, dt, kt * 128:(kt + 1) * 128],
                                wqkv_sb[:, 2, dt, :], start=dt == 0,
                                stop=dt == 3)
                        nc.vector.tensor_copy(
                            out=v_sb[:, kt, :, 0:DH],
                            in_=pvv[:].rearrange("p (h d) -> p h d", h=H))

                    ao = wrk.tile([128, 2, D], F32, tag="ao", bufs=1,
                                  name=f"ao_{l}")
                    for hp in range(H // 2):
                        qhT = qhT_l[hp]
                        khT = ab.tile([128, S], F32, tag="khT", bufs=1,
                                      name=f"khT_{nc.next_id()}")
                        for ch in range(2):
                            pk = mm_ps([128, 512])
                            for dt in range(4):
                                nc.tensor.matmul(
                                    pk[:],
                                    wqkv_sb[:, 1, dt, hp * 128:(hp + 1) * 128],
                                    hT_b[:, dt, ch * 512:(ch + 1) * 512],
                                    start=dt == 0, stop=dt == 3)
                            nc.scalar.copy(
                                khT[:, ch * 512:(ch + 1) * 512], pk[:])
                        for hh in range(2):
                            h = hp * 2 + hh
                            pa0 = mm_ps([128, DH + 1], pool=aops, tag="ao")
                            pa1 = mm_ps([128, DH + 1], pool=aops, tag="ao")
                            for kt in range(8):
                                pl = mm_ps([128, TPC])
                                nc.tensor.matmul(
                                    pl[:],
                                    khT[hh * 64:(hh + 1) * 64,
                                        kt * 128:(kt + 1) * 128],
                                    qhT[hh * 64:(hh + 1) * 64, :],
                                    start=True, stop=True)
                                pT = ab.tile([128, TPC], F32, tag="pT", bufs=2,
                                             name=f"pT_{nc.next_id()}")
                                nc.scalar.activation(pT[:], pl[:], AF.Exp)
                                nc.vector.tensor_tensor(
                                    out=pT[:], in0=pT[:], in1=indT[:, kt, :],
                                    op=ALU.mult)
                                for qt, pa in ((0, pa0), (1, pa1)):
                                    nc.tensor.matmul(
                                        pa[:], pT[:, qt * 128:(qt + 1) * 128],
                                        v_sb[:, kt, h, :], start=kt == 0,
                                        stop=kt == 7)
                            for qt, pa in ((0, pa0), (1, pa1)):
                                rec = sml.tile([128, 1], F32, tag="rec",
                                               name=f"rec_{nc.next_id()}")
                                nc.vector.reciprocal(rec[:], pa[:, DH:DH + 1])
                                nc.vector.tensor_scalar(
                                    out=ao[:, qt, h * DH:(h + 1) * DH],
                                    in0=pa[:, 0:DH], scalar1=rec[:],
                                    scalar2=None, op0=ALU.mult)
                    dump(f"ao{l}", ao[:])
                    aoT = ab.tile([128, 4, TPC], F32, tag="hT_own",
                                  name=f"aoT_{l}")
                    for qt in range(2):
                        for dt in range(4):
                            transpose_128(aoT[:, dt, qt * 128:(qt + 1) * 128],
                                          ao[:, qt, dt * 128:(dt + 1) * 128])
                    for qt in range(2):
                        po = mm_ps([128, D])
                        for dt in range(4):
                            nc.tensor.matmul(
                                po[:], aoT[:, dt, qt * 128:(qt + 1) * 128],
                                wqkv_sb[:, 3, dt, :], start=dt == 0,
                                stop=dt == 3)
                        nc.vector.tensor_tensor(
                            out=x_own[:, qt, :], in0=x_own[:, qt, :],
                            in1=po[:], op=ALU.add)
                dump(f"xattn{l}", x_own[:])

                # ---- MoE ----
                mh = wrk.tile([128, 2, D], F32, tag="xh", bufs=1,
                              name=f"mh_{l}")
                for qt in range(2):
                    normalize(x_own[:, qt, :], mh[:, qt, :])

                with (
                    tc.tile_pool(name=f"moe{l}", bufs=1) as mb,
                    tc.tile_pool(name=f"moeps{l}", bufs=2,
                                 space="PSUM") as mps,
                ):
                    rw_sb = mb.tile([128, 4, E], F32, tag="rw",
                                    name=f"rw_{l}")
                    nc.sync.dma_start(
                        rw_sb[:], rw_p[l].rearrange("(d p) n -> p d n", p=128))
                    mT_own = mb.tile([128, 4, TPC], F32, tag="mT_own",
                                     name=f"mTo_{l}")
                    for qt in range(2):
                        for dt in range(4):
                            transpose_128(
                                mT_own[:, dt, qt * 128:(qt + 1) * 128],
                                mh[:, qt, dt * 128:(dt + 1) * 128])

                    # split m into bf16 hi/lo and launch the expert
                    # AllGathers first; router + gate A2A overlap them
                    mhi = mb.tile([128, 2, D], BF16, tag="mhi",
                                  name=f"mhi_{l}")
                    nc.scalar.copy(mhi[:], mh[:])
                    mlo = mb.tile([128, 2, D], BF16, tag="mlo",
                                  name=f"mlo_{l}")
                    nc.vector.scalar_tensor_tensor(
                        out=mlo[:], in0=mh[:], scalar=1.0, in1=mhi[:],
                        op0=ALU.mult, op1=ALU.subtract)
                    mT_hi_own = mb.tile([128, 4, TPC], BF16, tag="mT_hi_own",
                                        name=f"mThio_{l}")
                    mT_lo_own = mb.tile([128, 4, TPC], BF16, tag="mT_lo_own",
                                        name=f"mTloo_{l}")
                    for qt in range(2):
                        for dt in range(4):
                            transpose_128(
                                mT_hi_own[:, dt, qt * 128:(qt + 1) * 128],
                                mhi[:, qt, dt * 128:(dt + 1) * 128],
                                dtype=BF16)
                    agh_in = dr.tile([D, TPC], BF16, tag="aghi",
                                     name=f"aghi_{l}")
                    nc.sync.dma_start(
                        agh_in.rearrange("(d p) t -> p d t", p=128),
                        mT_hi_own[:])
                    agh_out = dr.tile([NC * D, TPC], BF16,
                                      addr_space="Shared",
                                      tag="agho", name=f"agho_{l}")
                    nc.gpsimd.collective_compute(
                        "AllGather", ALU.bypass,
                        ins=[agh_in[:]], outs=[agh_out[:]],
                        replica_groups=[list(range(NC))])
                    for qt in range(2):
                        for dt in range(4):
                            transpose_128(
                                mT_lo_own[:, dt, qt * 128:(qt + 1) * 128],
                                mlo[:, qt, dt * 128:(dt + 1) * 128],
                                dtype=BF16)
                    agl_in = dr.tile([D, TPC], BF16, tag="agli",
                                     name=f"agli_{l}")
                    nc.sync.dma_start(
                        agl_in.rearrange("(d p) t -> p d t", p=128),
                        mT_lo_own[:])
                    agl_out = dr.tile([NC * D, TPC], BF16,
                                      addr_space="Shared",
                                      tag="aglo", name=f"aglo_{l}")
                    nc.gpsimd.collective_compute(
                        "AllGather", ALU.bypass,
                        ins=[agl_in[:]], outs=[agl_out[:]],
                        replica_groups=[list(range(NC))])

                    gate = wrk.tile([128, 2, E], F32, tag="gate", bufs=1,
                                    name=f"gate_{l}")
                    for qt in range(2):
                        pr = mm_ps([128, E])
                        for dt in range(4):
                            nc.tensor.matmul(
                                pr[:], mT_own[:, dt, qt * 128:(qt + 1) * 128],
                                rw_sb[:, dt, :], start=dt == 0, stop=dt == 3)
                        rl_ = sml.tile([128, E], F32, tag="rlog",
                                       name=f"rlog_{nc.next_id()}")
                        nc.vector.tensor_copy(out=rl_[:], in_=pr[:])
                        m1 = sml.tile([128, 1], F32, tag="m1",
                                      name=f"m1_{nc.next_id()}")
                        nc.vector.tensor_reduce(out=m1[:], in_=rl_[:],
                                                axis=AX.X, op=ALU.max)
                        t1 = sml.tile([128, E], F32, tag="t1",
                                      name=f"t1_{nc.next_id()}")
                        nc.vector.tensor_scalar(
                            out=t1[:], in0=rl_[:], scalar1=m1[:],
                            scalar2=None, op0=ALU.is_equal)
                        lp = sml.tile([128, E], F32, tag="lp",
                                      name=f"lp_{nc.next_id()}")
                        nc.vector.scalar_tensor_tensor(
                            out=lp[:], in0=t1[:], scalar=-1e30, in1=rl_[:],
                            op0=ALU.mult, op1=ALU.add)
                        m2 = sml.tile([128, 1], F32, tag="m2",
                                      name=f"m2_{nc.next_id()}")
                        nc.vector.tensor_reduce(out=m2[:], in_=lp[:],
                                                axis=AX.X, op=ALU.max)
                        dd = sml.tile([128, 1], F32, tag="dd",
                                      name=f"dd_{nc.next_id()}")
                        nc.vector.tensor_tensor(out=dd[:], in0=m1[:],
                                                in1=m2[:], op=ALU.subtract)
                        g1 = sml.tile([128, 1], F32, tag="g1",
                                      name=f"g1_{nc.next_id()}")
                        nc.scalar.activation(g1[:], dd[:], AF.Sigmoid)
                        g2 = sml.tile([128, 1], F32, tag="g2",
                                      name=f"g2_{nc.next_id()}")
                        nc.vector.tensor_scalar(
                            out=g2[:], in0=g1[:], scalar1=-1.0, scalar2=1.0,
                            op0=ALU.mult, op1=ALU.add)
                        t2 = sml.tile([128, E], F32, tag="t2",
                                      name=f"t2_{nc.next_id()}")
                        nc.vector.tensor_scalar(
                            out=t2[:], in0=lp[:], scalar1=m2[:], scalar2=None,
                            op0=ALU.is_equal)
                        nc.vector.tensor_scalar(
                            out=gate[:, qt, :], in0=t1[:], scalar1=g1[:],
                            scalar2=None, op0=ALU.mult)
                        nc.vector.scalar_tensor_tensor(
                            out=gate[:, qt, :], in0=t2[:], scalar=g2[:],
                            in1=gate[:, qt, :], op0=ALU.mult, op1=ALU.add)
                    dump(f"gate{l}", gate[:])

                    # gate columns to expert owners via AllToAll
                    gT = sml.tile([8, TPC], F32, tag="gT", name=f"gT_{l}")
                    for qt in range(2):
                        ptg = pst.tile([8, 128], F32, tag="tr",
                                       name=f"ptg_{nc.next_id()}")
                        nc.tensor.transpose(ptg[:], gate[:, qt, :], identf[:])
                        nc.vector.tensor_copy(
                            out=gT[:, qt * 128:(qt + 1) * 128], in_=ptg[:])
                    a2a_in = dr.tile([E, TPC], F32, tag="a2ai",
                                     name=f"a2ai_{l}")
                    nc.sync.dma_start(a2a_in[:], gT[:])
                    a2a_out = dr.tile([E, TPC], F32, tag="a2ao",
                                      name=f"a2ao_{l}")
                    nc.gpsimd.collective_compute(
                        "AllToAll", ALU.bypass, ins=[a2a_in[:]],
                        outs=[a2a_out[:]], replica_groups=[list(range(NC))])
                    ga = sml.tile([8, TPC], F32, tag="ga", name=f"ga_{l}")
                    nc.sync.dma_start(ga[:], a2a_out[:])
                    gcol = sml.tile([128, 2, E], F32, tag="gcol",
                                    name=f"gcol_{l}")
                    for hf in range(2):
                        ptg = pst.tile([128, 8], F32, tag="tr",
                                       name=f"ptg2_{nc.next_id()}")
                        nc.tensor.transpose(
                            ptg[:], ga[:, hf * 128:(hf + 1) * 128],
                            identf[0:8, 0:8])
                        nc.vector.tensor_copy(out=gcol[:, hf, :], in_=ptg[:])
                    dump(f"gcol{l}", gcol[:])

                    mT_hi = mb.tile([128, 4, NC, TPC], BF16, tag="mT_hi",
                                    name=f"mThi_{l}")
                    mT_lo = mb.tile([128, 4, NC, TPC], BF16, tag="mT_lo",
                                    name=f"mTlo_{l}")
                    for r in range(NC):
                        nc.sync.dma_start(
                            mT_hi[:, :, r, :],
                            agh_out[r * D:(r + 1) * D].rearrange(
                                "(d p) t -> p d t", p=128))
                        nc.sync.dma_start(
                            mT_lo[:, :, r, :],
                            agl_out[r * D:(r + 1) * D].rearrange(
                                "(d p) t -> p d t", p=128))

                    w1_sb = mb.tile([128, 2, 4, F], BF16, tag="w1",
                                    name=f"w1_{l}")
                    nc.sync.dma_start(
                        w1_sb[:],
                        w1_p[l].rearrange("s (d p) f -> p s d f", p=128))
                    w2_sb = mb.tile([128, 2, 16, D], BF16, tag="w2",
                                    name=f"w2_{l}")
                    nc.sync.dma_start(
                        w2_sb[:],
                        w2_p[l].rearrange("s (f p) d -> p s f d", p=128))

                    rs_in = dr.tile([T, D], F32, tag="rsi", name=f"rsi_{l}")
                    for tc4 in range(4):
                        h1hi = mb.tile([128, 16, 512], BF16, tag="h1hi",
                                       bufs=1, name=f"h1hi_{nc.next_id()}")
                        h1lo = mb.tile([128, 16, 512], BF16, tag="h1lo",
                                       bufs=1, name=f"h1lo_{nc.next_id()}")
                        rhs_hi = mT_hi[:, :, 2 * tc4:2 * tc4 + 2, :]
                        rhs_lo = mT_lo[:, :, 2 * tc4:2 * tc4 + 2, :]
                        for ft in range(16):
                            ph = mm_ps([128, 512],
                                       pool=mps if ft % 3 == 2 else None)
                            passes = []
                            for dt in range(4):
                                for si in (0, 1):
                                    passes.append(
                                        (w1_sb[:, si, dt,
                                               ft * 128:(ft + 1) * 128],
                                         rhs_hi[:, dt]))
                            for dt in range(4):
                                passes.append(
                                    (w1_sb[:, 0, dt,
                                           ft * 128:(ft + 1) * 128],
                                     rhs_lo[:, dt]))
                            for i, (wsl, rh) in enumerate(passes):
                                nc.tensor.matmul(
                                    ph[:], wsl,
                                    rh.rearrange("p r t -> p (r t)"),
                                    start=i == 0, stop=i == 11)
                            h1f = wrk.tile([128, 512], F32, tag="h1f",
                                           name=f"h1f_{nc.next_id()}")
                            nc.scalar.activation(h1f[:], ph[:],
                                                 AF.Gelu_apprx_tanh)
                            nc.scalar.copy(h1hi[:, ft, :], h1f[:])
                            nc.vector.scalar_tensor_tensor(
                                out=h1lo[:, ft, :], in0=h1f[:], scalar=1.0,
                                in1=h1hi[:, ft, :], op0=ALU.mult,
                                op1=ALU.subtract)
                        for qs in range(4):
                            g = tc4 * 4 + qs
                            ph2 = mm_ps([128, D])
                            nmm = 0
                            for ft in range(16):
                                for hsl, wsl in (
                                    (h1hi[:, ft, qs * 128:(qs + 1) * 128],
                                     w2_sb[:, 0, ft, :]),
                                    (h1hi[:, ft, qs * 128:(qs + 1) * 128],
                                     w2_sb[:, 1, ft, :]),
                                    (h1lo[:, ft, qs * 128:(qs + 1) * 128],
                                     w2_sb[:, 0, ft, :]),
                                ):
                                    nc.tensor.matmul(
                                        ph2[:], hsl, wsl, start=nmm == 0,
                                        stop=nmm == 47)
                                    nmm += 1
                            yt = wrk.tile([128, D], F32, tag="yt",
                                          name=f"yt_{nc.next_id()}")
                            nc.vector.tensor_scalar(
                                out=yt[:], in0=ph2[:],
                                scalar1=gcol[:, g % 2, g // 2:g // 2 + 1],
                                scalar2=None, op0=ALU.mult)
                            nc.sync.dma_start(
                                rs_in[g * 128:(g + 1) * 128, :], yt[:])
                    rs_out = dr.tile([TPC, D], F32, tag="rso",
                                     name=f"rso_{l}")
                    nc.gpsimd.collective_compute(
                        "ReduceScatter", ALU.add, ins=[rs_in[:]],
                        outs=[rs_out[:]], replica_groups=[list(range(NC))])
                    y_own = wrk.tile([128, 2, D], F32, tag="y_own", bufs=1,
                                     name=f"yo_{l}")
                    nc.sync.dma_start(
                        y_own[:], rs_out.rearrange("(t p) d -> p t d", p=128))
                    dump(f"yown{l}", y_own[:])
                    for qt in range(2):
                        nc.vector.tensor_tensor(
                            out=x_own[:, qt, :], in0=x_own[:, qt, :],
                            in1=y_own[:, qt, :], op=ALU.add)
                dump(f"xmoe{l}", x_own[:])

            # =======================================================
            # final LN + vocab projection (column-sharded, bf16)
            # =======================================================
            with tc.tile_pool(name="voc", bufs=1) as vb:
                xf = wrk.tile([128, 2, D], F32, tag="xh", bufs=1, name="xf")
                for qt in range(2):
                    normalize(x_own[:, qt, :], xf[:, qt, :])
                xfb = vb.tile([128, 2, D], BF16, tag="xfb", name="xfb")
                nc.scalar.copy(xfb[:], xf[:])
                xfT_own = vb.tile([128, 4, TPC], BF16, tag="xfT", name="xfT")
                for qt in range(2):
                    for dt in range(4):
                        transpose_128(xfT_own[:, dt, qt * 128:(qt + 1) * 128],
                                      xfb[:, qt, dt * 128:(dt + 1) * 128],
                                      dtype=BF16)
                # AllGather in two token halves: vocab matmuls for the
                # first half overlap the second transfer
                xfT_full = vb.tile([128, 4, NC, TPC], BF16, tag="xfT_full",
                                   name="xfTf")
                for half in range(2):
                    agi = dr.tile([D, 128], BF16, tag=f"ag3i{half}",
                                  name=f"ag3i{half}")
                    nc.sync.dma_start(
                        agi.rearrange("(d p) t -> p d t", p=128),
                        xfT_own[:, :, half * 128:(half + 1) * 128])
                    ago = dr.tile([NC * D, 128], BF16, addr_space="Shared",
                                  tag=f"ag3o{half}", name=f"ag3o{half}")
                    nc.gpsimd.collective_compute(
                        "AllGather", ALU.bypass, ins=[agi[:]],
                        outs=[ago[:]], replica_groups=[list(range(NC))])
                    for r in range(NC):
                        nc.sync.dma_start(
                            xfT_full[:, :, r,
                                     half * 128:(half + 1) * 128],
                            ago[r * D:(r + 1) * D].rearrange(
                                "(d p) t -> p d t", p=128))

                NVC = 8
                CW = VPC // NVC  # 500
                for vc in range(NVC):
                    owc = vb.tile([128, 4, CW], BF16, tag="outw", bufs=2,
                                  name=f"owc_{vc}")
                    nc.sync.dma_start(
                        owc[:],
                        outw_p[:, vc * CW:(vc + 1) * CW].rearrange(
                            "(d p) v -> p d v", p=128))
                    for qt in (list(range(0, 16, 2))
                               + list(range(1, 16, 2))):
                        r, hf = qt // 2, qt % 2
                        pv = mm_ps([128, CW])
                        for dt in range(4):
                            nc.tensor.matmul(
                                pv[:],
                                xfT_full[:, dt, r, hf * 128:(hf + 1) * 128],
                                owc[:, dt, :], start=dt == 0, stop=dt == 3)
                        oc = vb.tile([128, CW], F32, tag="oc", bufs=3,
                                     name=f"oc_{nc.next_id()}")
                        if qt % 2 == 0:
                            nc.vector.tensor_copy(out=oc[:], in_=pv[:])
                        else:
                            nc.scalar.copy(oc[:], pv[:])
                        nc.sync.dma_start(
                            out_p[qt * 128:(qt + 1) * 128,
                                  vc * CW:(vc + 1) * CW], oc[:])

    nc.compile()
    return nc


# -------------------------------------------------------------- host side --
_CACHE = {}
_LAST_IN_MAPS = None


def _np(x, dt=np.float32):
    return np.ascontiguousarray(np.asarray(x), dtype=dt)


def kernel(**inputs):
    ids = _np(inputs["input_ids"], np.int64).reshape(B, S)
    tok_emb = _np(inputs["tok_emb"])
    pos_emb = _np(inputs["pos_emb"])
    ln1_g, ln1_b = _np(inputs["ln1_g"]), _np(inputs["ln1_b"])
    ln2_g, ln2_b = _np(inputs["ln2_g"]), _np(inputs["ln2_b"])
    lnf_g, lnf_b = _np(inputs["lnf_g"]), _np(inputs["lnf_b"])
    idx_qw, idx_qb = _np(inputs["idx_qw"]), _np(inputs["idx_qb"])
    idx_kw, idx_kb = _np(inputs["idx_kw"]), _np(inputs["idx_kb"])
    idx_hw = _np(inputs["idx_hw"])
    wq, bq = _np(inputs["wq"]), _np(inputs["bq"])
    wk, bk = _np(inputs["wk"]), _np(inputs["bk"])
    wv, bv = _np(inputs["wv"]), _np(inputs["bv"])
    wo, bo = _np(inputs["wo"]), _np(inputs["bo"])
    router_w, router_b = _np(inputs["router_w"]), _np(inputs["router_b"])
    e_w1, e_b1 = _np(inputs["e_w1"]), _np(inputs["e_b1"])
    e_w2, e_b2 = _np(inputs["e_w2"]), _np(inputs["e_b2"])
    out_w, out_b = _np(inputs["out_w"]), _np(inputs["out_b"])

    for nm, b in [("ln1_b", ln1_b), ("ln2_b", ln2_b), ("lnf_b", lnf_b),
                  ("idx_qb", idx_qb), ("idx_kb", idx_kb), ("bq", bq),
                  ("bk", bk), ("bv", bv), ("bo", bo), ("router_b", router_b),
                  ("e_b1", e_b1), ("e_b2", e_b2), ("out_b", out_b)]:
        assert np.abs(b).max() == 0.0, f"nonzero bias {nm} unsupported"

    x0 = tok_emb[ids.reshape(-1)] + np.tile(pos_emb[:S], (B, 1))  # [T, D]

    scale = 1.0 / np.sqrt(DH)
    idxq_f = idx_qw * ln1_g[:, :, None]
    signs = np.sign(idx_hw)
    signs[signs == 0] = 1.0
    for l in range(L):
        for h in range(HI):
            idxq_f[l][:, h * DI:(h + 1) * DI] *= abs(idx_hw[l, h])
    idxk_f = idx_kw * ln1_g[:, :, None]
    wq_f = wq * ln1_g[:, :, None] * scale
    wk_f = wk * ln1_g[:, :, None]
    wv_f = wv * ln1_g[:, :, None]
    wqkv = np.stack([wq_f, wk_f, wv_f, wo], axis=1)  # [L, 4, D, D]
    rw_f = router_w * ln2_g[:, :, None]
    w1_f = e_w1 * ln2_g[:, None, :, None]            # [L, E, D, F]
    outw_f = out_w * lnf_g[:, None]

    def split_pair(w):
        hi = w.astype(bf16)
        lo = (w - hi.astype(np.float32)).astype(bf16)
        return np.ascontiguousarray(np.stack([hi, lo], axis=0))

    w1_pair = [split_pair(np.ascontiguousarray(w1_f[:, c]).astype(np.float32)
                          ).transpose(1, 0, 2, 3)
               for c in range(NC)]   # [L, 2, D, F]
    w2_pair = [split_pair(np.ascontiguousarray(e_w2[:, c]).astype(np.float32)
                          ).transpose(1, 0, 2, 3)
               for c in range(NC)]   # [L, 2, F, D]
    w1_pair = [np.ascontiguousarray(w) for w in w1_pair]
    w2_pair = [np.ascontiguousarray(w) for w in w2_pair]

    if "nc" not in _CACHE:
        _CACHE["nc"] = _build(signs)
    nc = _CACHE["nc"]

    ident_b = np.eye(128, dtype=bf16)
    ident_f = np.eye(128, dtype=np.float32)
    in_maps = []
    for c in range(NC):
        rows = slice(c * TPC, (c + 1) * TPC)
        p = np.arange(S)[(c % 4) * TPC:(c % 4 + 1) * TPC]
        c01 = (np.arange(S)[None, :] <= p[:, None]).astype(np.float32)
        in_maps.append({
            "x0": x0[rows].astype(np.float32),
            "c01": c01,
            "idxq": idxq_f.astype(np.float32),
            "idxk": idxk_f.astype(np.float32),
            "wqkv": wqkv.astype(np.float32),
            "rw": rw_f.astype(np.float32),
            "w1": w1_pair[c],
            "w2": w2_pair[c],
            "outw": outw_f[:, c * VPC:(c + 1) * VPC].astype(bf16),
            "idb": ident_b,
            "idf": ident_f,
        })

    global _LAST_IN_MAPS
    _LAST_IN_MAPS = in_maps
    res = run_bass_kernel_spmd(nc, in_maps, core_ids=list(range(NC)))
    outs = [res.results[c]["out"] for c in range(NC)]
    full = np.concatenate(outs, axis=1).reshape(B, S, V)
    return np.ascontiguousarray(full, dtype=np.float32)


if __name__ == "__main__":
    import reference
    inp = {k: np.asarray(v) for k, v in reference.setup_inputs().items()}
    got = kernel(**inp)
    print("kernel output", got.shape, got.dtype)
